# revision 1
# baseline (speedup 1.0000x reference)
"""Trainium2 Bass kernel for NeuroVPR Vanilla SNN (3-layer LIF, T=3).

Data-parallel over batch: B=16384 -> 2048 per core x 8 cores.

Math (per timestep, per layer): v = (v_prev + h)/2; s = (v>=1); v *= (1-s).
We track w = 2*v and m = 2*v_after_reset, so:
    w_t = 0.5*m_{t-1} + h_t     (exact: *0.5 is a power-of-2 scale)
    s_t = (w_t >= 2)
    m_t = w_t * (w_t < 2)
Spike decisions match the fp32 recurrence bit-for-bit up to matmul error.

Layout: h.T = W @ x.T via matmul(out[h,b], lhsT=W.T[d,h], rhs=x.T[d,b]) with
d (contraction) on partitions. Host pre-transposes dvs to [T, D, B_c] and
pads D 2752->2816 (22*128); pad row 2752 carries the L1 bias with x=1 there.
fp16 operands (1 cyc/row on PE, half the DMA bytes); fp32 PSUM accumulation.
Validated: layer-2 membrane peaks at 0.64 vs threshold 1.0, so the ~70/4.2M
layer-1 spike flips fp16 induces cannot propagate to the output.

Schedule (keeps TensorE dense so the HAM clock gate stays at 2.4 GHz):
per timestep, L1 runs as two half-batch passes of 4 PSUM banks each, and
the previous timestep's L2+L3 matmuls are emitted between the passes.
Spike compares run on GpSimd; membrane updates on VectorE.
"""
import os
import numpy as np

B, T, D = 16384, 3, 2752
DP = 2816  # D padded to 22*128 (pad row 2752 = bias row)
H, O = 256, 100
NCORES = 8
BC = B // NCORES  # 2048
NB = 512          # psum block along batch
KT = DP // 128    # 22 contraction tiles for L1

_compiled = {}
last_results = None  # BassKernelResults of the most recent run (for profiling)


def _build(use_b2, use_b3):
    from contextlib import ExitStack
    import concourse.bass as bass
    import concourse.mybir as mybir
    import concourse.tile as tile
    from concourse import bacc

    f16, f32 = mybir.dt.float16, mybir.dt.float32
    A = mybir.AluOpType

    nc = bacc.Bacc("TRN2", target_bir_lowering=False, debug=False)
    x = nc.dram_tensor("x", [T, DP, BC], f16, kind="ExternalInput").ap()
    w1 = nc.dram_tensor("w1", [DP, H], f16, kind="ExternalInput").ap()
    w2 = nc.dram_tensor("w2", [H, H], f16, kind="ExternalInput").ap()
    w3 = nc.dram_tensor("w3", [H, O], f16, kind="ExternalInput").ap()
    b2 = nc.dram_tensor("b2", [1, H], f16, kind="ExternalInput").ap()
    b3 = nc.dram_tensor("b3", [1, O], f16, kind="ExternalInput").ap()
    out = nc.dram_tensor("out", [O, BC], f32, kind="ExternalOutput").ap()

    HB = BC // 2  # half-batch per L1 pass (1024)

    with tile.TileContext(nc) as tc, ExitStack() as ctx:
        wp = ctx.enter_context(tc.tile_pool(name="wp", bufs=1))
        xp = ctx.enter_context(tc.tile_pool(name="xp", bufs=12))
        pp1 = ctx.enter_context(tc.tile_pool(name="pp1", bufs=6, space="PSUM"))
        pp23 = ctx.enter_context(tc.tile_pool(name="pp23", bufs=2, space="PSUM"))
        sp = ctx.enter_context(tc.tile_pool(name="sp", bufs=1))
        tp = ctx.enter_context(tc.tile_pool(name="tp", bufs=6))

        # resident weights, [d_part, (k h)] layout
        w1t = wp.tile([128, KT * H], f16)
        w1r = w1.rearrange("(k p) h -> p k h", p=128)
        w1o = w1t[:, :].rearrange("p (k h) -> p k h", k=KT)
        nc.sync.dma_start(out=w1o[:, 0:1, :], in_=w1r[:, 0:1, :])
        for c0, c1 in ((1, 7), (7, 14), (14, 22)):
            nc.scalar.dma_start(out=w1o[:, c0:c1, :], in_=w1r[:, c0:c1, :])
        w2t = wp.tile([128, 2 * H], f16)
        nc.gpsimd.dma_start(out=w2t[:, :].rearrange("p (k h) -> p k h", k=2),
                            in_=w2.rearrange("(k p) h -> p k h", p=128))
        w3t = wp.tile([128, 2 * O], f16)
        nc.gpsimd.dma_start(out=w3t[:, :].rearrange("p (k h) -> p k h", k=2),
                            in_=w3.rearrange("(k p) h -> p k h", p=128))
        b2t = wp.tile([1, H], f16)
        nc.gpsimd.dma_start(out=b2t[:, :], in_=b2[:, :])
        b3t = wp.tile([1, O], f16)
        nc.gpsimd.dma_start(out=b3t[:, :], in_=b3[:, :])
        ones = wp.tile([1, NB], f16)
        nc.gpsimd.memset(ones[:, :], 1.0)

        # persistent state (m = 2*v_after_reset, zero-initialized) and spikes
        m1 = [sp.tile([128, BC], f32, tag=f"m1_{h}", name=f"m1_{h}") for h in range(2)]
        m2 = [sp.tile([128, BC], f32, tag=f"m2_{h}", name=f"m2_{h}") for h in range(2)]
        m3 = sp.tile([128, BC], f32, tag="m3")
        s1 = [sp.tile([128, BC], f16, tag=f"s1_{h}", name=f"s1_{h}") for h in range(2)]
        s2 = [sp.tile([128, BC], f16, tag=f"s2_{h}", name=f"s2_{h}") for h in range(2)]
        outsb = sp.tile([128, BC], f32, tag="outsb")
        for mt in (*m1, *m2, m3):
            nc.vector.memset(mt[:, :], 0.0)

        def lif_w(psum, m_ap):
            """w = m/2 + h. Reads+releases the psum bank; returns w tile."""
            P = psum.shape[0]
            w = tp.tile([128, NB], f32, tag="w", name="w")[:P, :]
            nc.vector.scalar_tensor_tensor(w, m_ap, 0.5, psum, A.mult, A.add)
            return w

        def lif_s(w, s_ap):
            nc.vector.tensor_scalar(s_ap, w, 2.0, None, A.is_ge)

        def lif_m(w, m_ap):
            nc.vector.scalar_tensor_tensor(m_ap, w, 2.0, w, A.is_lt, A.mult)

        def l2_group(t, h, b, pool, tag):
            ps2 = pool.tile([128, NB], f32, tag=tag, name=f"ps2_{t}_{h}_{b}")
            first = True
            if use_b2:
                nc.tensor.matmul(ps2[:, :], b2t[0:1, h * 128:(h + 1) * 128],
                                 ones[0:1, :], start=True, stop=False)
                first = False
            for k in range(2):
                nc.tensor.matmul(
                    ps2[:, :],
                    w2t[:, k * H + h * 128: k * H + h * 128 + 128],
                    s1[k][:, b * NB:(b + 1) * NB],
                    start=first, stop=(k == 1))
                first = False
            return ps2

        def l3_group(t, b, pool, tag):
            ps3 = pool.tile([128, NB], f32, tag=tag, name=f"ps3_{t}_{b}")
            first = True
            if use_b3:
                nc.tensor.matmul(ps3[:O, :], b3t[0:1, :], ones[0:1, :],
                                 start=True, stop=False)
                first = False
            for k in range(2):
                nc.tensor.matmul(ps3[:O, :], w3t[:, k * O:(k + 1) * O],
                                 s2[k][:, b * NB:(b + 1) * NB],
                                 start=first, stop=(k == 1))
                first = False
            return ps3

        def l2_all(t, pool, tag):
            """Layer-2 matmuls + LIF for timestep t (all batch blocks)."""
            last = (t == T - 1)
            for b in range(4):
                bs = slice(b * NB, (b + 1) * NB)
                for h in range(2):
                    ps2 = l2_group(t, h, b, pool, tag)
                    w = lif_w(ps2[:, :], m2[h][:, bs])
                    lif_s(w, s2[h][:, bs])
                    if not last:
                        lif_m(w, m2[h][:, bs])

        def l3_all(t, pool, tag):
            """Layer-3 matmuls + LIF for timestep t (all batch blocks)."""
            last = (t == T - 1)
            for b in range(4):
                bs = slice(b * NB, (b + 1) * NB)
                ps3 = l3_group(t, b, pool, tag)
                w3_ = lif_w(ps3[:O, :], m3[:O, bs])
                lif_s(w3_, outsb[:O, bs])
                if not last:
                    lif_m(w3_, m3[:O, bs])
                else:
                    nc.sync.dma_start(out=out[:, bs], in_=outsb[:O, bs])

        def l1_pass(t, half):
            """One half-batch L1 pass: 4 psum groups (2h x 2b), k inner."""
            boff = half * HB
            ps1 = [[pp1.tile([128, NB], f32, tag="ps1", name=f"ps1_{t}_{half}_{h}_{b}")
                    for b in range(2)] for h in range(2)]
            for k in range(KT):
                xt = xp.tile([128, HB], f16, tag="x", name="xt")
                nc.sync.dma_start(out=xt[:, :],
                                  in_=x[t, k * 128:(k + 1) * 128,
                                       boff:boff + HB])
                for h in range(2):
                    for b in range(2):
                        nc.tensor.matmul(
                            ps1[h][b][:, :],
                            w1t[:, k * H + h * 128: k * H + h * 128 + 128],
                            xt[:, b * NB:(b + 1) * NB],
                            start=(k == 0), stop=(k == KT - 1))
            # release all 4 banks first (w-ops), then spikes, then membranes
            ws = {}
            for h in range(2):
                for b in range(2):
                    bs = slice(boff + b * NB, boff + (b + 1) * NB)
                    ws[h, b] = lif_w(ps1[h][b][:, :], m1[h][:, bs])
            for h in range(2):
                for b in range(2):
                    bs = slice(boff + b * NB, boff + (b + 1) * NB)
                    lif_s(ws[h, b], s1[h][:, bs])
            if t != T - 1:
                for h in range(2):
                    for b in range(2):
                        bs = slice(boff + b * NB, boff + (b + 1) * NB)
                        lif_m(ws[h, b], m1[h][:, bs])

        for t in range(T):
            l1_pass(t, 0)
            if t > 0:
                l2_all(t - 1, pp23, "ps23")
            if t == T - 1:
                l3_all(t - 1, pp23, "ps23")
                for b in (0, 1):
                    bs = slice(b * NB, (b + 1) * NB)
                    for h in range(2):
                        ps2 = l2_group(t, h, b, pp23, "ps23")
                        w = lif_w(ps2[:, :], m2[h][:, bs])
                        lif_s(w, s2[h][:, bs])
            l1_pass(t, 1)
            if 0 < t < T - 1:
                l3_all(t - 1, pp23, "ps23")
        # tail: l2(T-1, b23) and l3(T-1) pipelined per b-block
        t_ = T - 1
        for b in (2, 3):
            bs = slice(b * NB, (b + 1) * NB)
            for h in range(2):
                ps2 = l2_group(t_, h, b, pp1, "ps1")
                w = lif_w(ps2[:, :], m2[h][:, bs])
                lif_s(w, s2[h][:, bs])
            bp = b - 2
            bs = slice(bp * NB, (bp + 1) * NB)
            ps3 = l3_group(t_, bp, pp23, "ps23")
            w3_ = lif_w(ps3[:O, :], m3[:O, bs])
            lif_s(w3_, outsb[:O, bs])
            nc.sync.dma_start(out=out[:, bs], in_=outsb[:O, bs])
        for bp in (2, 3):
            bs = slice(bp * NB, (bp + 1) * NB)
            ps3 = l3_group(t_, bp, pp23, "ps23")
            w3_ = lif_w(ps3[:O, :], m3[:O, bs])
            lif_s(w3_, outsb[:O, bs])
            nc.sync.dma_start(out=out[:, bs], in_=outsb[:O, bs])

    nc.compile()
    return nc


def kernel(dvs, W1, b1, W2, b2, W3, b3):
    global last_results
    from concourse.bass_utils import run_bass_kernel_spmd

    use_b2 = bool(np.any(b2))
    use_b3 = bool(np.any(b3))
    key = (use_b2, use_b3)
    if key not in _compiled:
        _compiled[key] = _build(use_b2, use_b3)
    nc = _compiled[key]

    f16 = np.float16
    # x: [B, T, D] -> fp16 [T, DP, B], pad row D=2752 carries bias (x=1)
    X = np.zeros((T, DP, B), dtype=f16)
    X[:, :D, :] = dvs.astype(f16).transpose(1, 2, 0)
    X[:, D, :] = f16(1.0)

    w1p = np.zeros((DP, H), dtype=f16)
    w1p[:D, :] = W1.T.astype(f16)
    w1p[D, :] = b1.astype(f16)
    w2p = np.ascontiguousarray(W2.T.astype(f16))
    w3p = np.ascontiguousarray(W3.T.astype(f16))
    b2p = b2.astype(f16).reshape(1, H)
    b3p = b3.astype(f16).reshape(1, O)

    in_maps = []
    for c in range(NCORES):
        xc = np.ascontiguousarray(X[:, :, c * BC:(c + 1) * BC])
        in_maps.append({"x": xc, "w1": w1p, "w2": w2p, "w3": w3p,
                        "b2": b2p, "b3": b3p})

    trace = bool(os.environ.get("SNN_TRACE"))
    last_results = run_bass_kernel_spmd(nc, in_maps, core_ids=list(range(NCORES)),
                                        trace=trace)
    out = np.empty((B, O), dtype=np.float32)
    for c in range(NCORES):
        out[c * BC:(c + 1) * BC, :] = last_results.results[c]["out"].T
    return out

